# revision 1
# baseline (speedup 1.0000x reference)
"""GAT-style attention kernel for Trainium2, 8 NeuronCores.

Reference computation (N=M=8192, D=256, f32):
    e1 = input1 @ a1; e2 = (input2 @ a2).T
    e  = leaky_relu(e1 + e2, 0.2)
    att = softmax(where(adj>0, e, -9e15), axis=1)
    att = att * adj.sum(1, keepdims=True)
    att = att*0.5 + adj*0.5
    out = att @ input2

Device math per row i (w_ij = exp(leaky_relu(e1_i + e2_j))):
    denom_i = sum_j adj_ij w_ij ; deg_i = sum_j adj_ij ; delta_i = deg_i/denom_i
    out_i = 0.5 * [ (delta_i * (adj.w) + adj) @ input2 ]_i

Sharding: rows of N across 8 cores (1024 each); input2/a1/a2 replicated.

Per-core pipeline, 8 blocks of (128 rows x 8192 cols):
    adjb = bf16(adj block)                     [SWDGE cast-DMA]
    m    = adjb - 1, accum+8192 -> deg         [DVE TS 4x + accum]
    t    = m + e2b', e2b' = e2/BIG (bf16)      [DVE TT 2x, in-place on m]
    lr   = prelu(BIG*t + e1_i, 0.2)            [ACT f32; BIG*(adj-1) masks]
    num  = exp(lr), accum -> denom             [ACT, bf16 out]
    s    = num * delta_i                       [DVE TS 4x, in-place]
    att  = s + adjb                            [Pool TT, in-place on s]
    attT chunks via PE transpose (exact)       [PE, PSUM bf16]
    att_sb = copy(attT)                        [DVE PSUM->SBUF 2x]
    acc += att_sb_chunk.T @ input2_bf16_chunk  [PE matmul]
    out = 0.5 * acc                            [DVE]
"""

import os
import numpy as np

import concourse.bass as bass
import concourse.bacc as bacc
import concourse.tile as tile
from concourse import mybir
from concourse.bass_utils import run_bass_kernel_spmd

try:
    import ml_dtypes

    _BF16_NP = ml_dtypes.bfloat16
except Exception:  # pragma: no cover
    _BF16_NP = None

N, M, D = 8192, 8192, 256
NCORES = 8
ROWS = N // NCORES
P = 128
NBLK = ROWS // P  # 8
NCHUNK = M // P  # 64
BIG = 150.0
SLOPE = 0.2
GRP = 8  # transpose chunks per PSUM staging bank

F32 = mybir.dt.float32
BF16 = mybir.dt.bfloat16
F16 = mybir.dt.float16

LAST_EXEC_NS = None
_CACHED = None


def _build_kernel(reps=1):
    nc = bacc.Bacc("TRN2", target_bir_lowering=False, debug=False)

    inp1 = nc.dram_tensor("input1", [ROWS, D], F32, kind="ExternalInput").ap()
    inp2 = nc.dram_tensor("input2", [M, D], F32, kind="ExternalInput").ap()
    adj = nc.dram_tensor("adj", [ROWS, M], F32, kind="ExternalInput").ap()
    a1b = nc.dram_tensor("a1b", [P, D], F32, kind="ExternalInput").ap()
    a2b = nc.dram_tensor("a2b", [P, D], F32, kind="ExternalInput").ap()
    identd = nc.dram_tensor("identd", [P, P], F16, kind="ExternalInput").ap()
    out = nc.dram_tensor("out", [ROWS, D], F32, kind="ExternalOutput").ap()

    # DRAM bounce for flattening e2 (computed column-wise) into row order
    e2d = nc.dram_tensor("e2d", [1, M], F32).ap()

    AL = mybir.AluOpType

    with tile.TileContext(nc) as tc:
        with (
            tc.tile_pool(name="persist", bufs=1) as persist,
            tc.tile_pool(name="setup", bufs=3) as setup,
            tc.tile_pool(name="small", bufs=4) as small,
            tc.tile_pool(name="adjp", bufs=3) as adjp,
            tc.tile_pool(name="mtp", bufs=2) as mtp,
            tc.tile_pool(name="lrp", bufs=1) as lrp,
            tc.tile_pool(name="nump", bufs=2) as nump,
            tc.tile_pool(name="attp", bufs=2) as attp,
            tc.tile_pool(name="outp", bufs=2) as outp,
            tc.tile_pool(name="psA", bufs=3, space="PSUM") as psA,
            tc.tile_pool(name="psO", bufs=2, space="PSUM") as psO,
        ):
            # ---------------- setup ----------------
            ident = persist.tile([P, P], F16)
            nc.sync.dma_start(ident[:], identd[:])
            a1t = persist.tile([P, D], F32)
            nc.sync.dma_start(a1t[:], a1b[:])
            a2t = persist.tile([P, D], F32)
            nc.sync.dma_start(a2t[:], a2b[:])

            inp2b = persist.tile([P, NCHUNK * D], F16)
            e2col = persist.tile([P, NCHUNK], F32)
            e1col = persist.tile([P, NBLK], F32)
            adjb_t = {}
            # batched loads: one (128, 8*D) strided DMA covers 8 row-tiles
            TB = 4  # tiles per batched DMA
            for g in range(NCHUNK // TB):
                tmp = setup.tile([P, TB * D], F32, tag="itile")
                src = inp2[g * TB * P : (g + 1) * TB * P, :]
                src = src.rearrange("(tb p) d -> p tb d", p=P, tb=TB)
                dst = tmp[:].rearrange("p (tb d) -> p tb d", tb=TB, d=D)
                nc.sync.dma_start(dst, src)
                nc.gpsimd.tensor_copy(inp2b[:, g * TB * D : (g + 1) * TB * D], tmp[:])
                for k in range(TB):
                    t = g * TB + k
                    scr = small.tile([P, 1], F32, tag="amrdummy")
                    nc.vector.affine_mul_reduce(
                        out=scr[:].broadcast_to([P, D]),
                        accum_out=e2col[:, t : t + 1],
                        in0=tmp[:, k * D : (k + 1) * D],
                        in1=a2t[:],
                        scale=1.0 / BIG,
                        bias=0.0,
                    )
            # adj block 0 (gates m(0))
            adjb_t[0] = adjp.tile([P, M], BF16, name="adjb", tag="adjb")
            nc.gpsimd.dma_start(adjb_t[0][:], adj[0:P, :])
            # e1 (P1 bias), two TB-sized batches
            for g1 in range(NBLK // TB):
                tmp1 = setup.tile([P, TB * D], F32, tag="itile")
                src1 = inp1[g1 * TB * P : (g1 + 1) * TB * P, :].rearrange(
                    "(tb p) d -> p tb d", p=P, tb=TB
                )
                dst1 = tmp1[:].rearrange("p (tb d) -> p tb d", tb=TB, d=D)
                nc.sync.dma_start(dst1, src1)
                for k1 in range(TB):
                    b1 = g1 * TB + k1
                    scr = small.tile([P, 1], F32, tag="amrdummy")
                    nc.vector.affine_mul_reduce(
                        out=scr[:].broadcast_to([P, D]),
                        accum_out=e1col[:, b1 : b1 + 1],
                        in0=tmp1[:, k1 * D : (k1 + 1) * D],
                        in1=a1t[:],
                        scale=1.0,
                        bias=0.0,
                    )
            # e2col[p, t] = e2[t*128+p]/BIG -> e2d[j] (scatter via stride AP)
            e2d_scat = e2d.rearrange("one (t p) -> one p t", p=P, t=NCHUNK)
            nc.sync.dma_start(e2d_scat[0], e2col[:])

            # e2b' = broadcast(e2/BIG) to all partitions, f16
            e2b = persist.tile([P, M], F16)
            nc.gpsimd.dma_start(e2b[:], e2d[:].broadcast_to([P, M]))

            # adj block 1
            adjb_t[1] = adjp.tile([P, M], BF16, name="adjb", tag="adjb")
            nc.gpsimd.dma_start(adjb_t[1][:], adj[P : 2 * P, :])

            # ---------------- main loop (software-pipelined, 1-block skew) ----
            # iter b: front(b) = m,t,P1,P2 ; back(b-1) = delta,s,att,PE,out
            HD = 6144   # DVE columns of the TT splits; Pool gets the rest
            mt_t, num_t, deg_t, den_t = {}, {}, {}, {}

            def emit_front(b):
                # adjb DMA for b+1 is issued in back(b-1) below
                mt = mtp.tile([P, M], F16, tag="mt")
                mt_t[b] = mt
                deg = small.tile([P, 1], F32, tag="deg")
                deg_t[b] = deg
                nc.vector.tensor_scalar(
                    mt[:], adjb_t[b][:], -1.0, float(M), AL.add, AL.add,
                    accum_out=deg[:],
                )
                nc.vector.tensor_tensor(mt[:, :HD], mt[:, :HD], e2b[:, :HD], AL.add)
                nc.gpsimd.tensor_tensor(mt[:, HD:], mt[:, HD:], e2b[:, HD:], AL.add)
                lr = lrp.tile([P, M], F16)
                nc.scalar.activation(
                    lr[:], mt[:], mybir.ActivationFunctionType.Prelu,
                    bias=e1col[:, b : b + 1], scale=BIG, alpha=SLOPE,
                )
                num = nump.tile([P, M], BF16, tag="num")
                num_t[b] = num
                den = small.tile([P, 1], F32, tag="denom")
                den_t[b] = den
                nc.scalar.activation(
                    num[:], lr[:], mybir.ActivationFunctionType.Exp,
                    accum_out=den[:],
                )

            def emit_back(b):
                rec = small.tile([P, 1], F32, tag="rec")
                nc.vector.reciprocal(rec[:], den_t[b][:])
                delta = small.tile([P, 1], F32, tag="delta")
                nc.vector.tensor_tensor(delta[:], deg_t[b][:], rec[:], AL.mult)
                satt = mtp.tile([P, M], F16, tag="mt")
                adjb = adjb_t.pop(b)
                # early prefetch (adjp has 3 bufs: b, b+1, b+2)
                if b + 2 < NBLK:
                    nx = adjp.tile([P, M], BF16, name="adjb", tag="adjb")
                    adjb_t[b + 2] = nx
                    nc.gpsimd.dma_start(nx[:], adj[(b + 2) * P : (b + 3) * P, :])
                acc = psO.tile([P, D], F32)
                GW = GRP * P  # 1024 columns per group
                H1 = 512     # DVE part of each group's att add; Pool gets 512
                for g in range(NCHUNK // GRP):
                    g0 = g * GW
                    sg = satt[:, g0 : g0 + GW]
                    nc.vector.tensor_scalar(
                        sg, num_t[b][:, g0 : g0 + GW], delta[:], None, AL.mult
                    )
                    nc.vector.tensor_tensor(
                        satt[:, g0 : g0 + H1], satt[:, g0 : g0 + H1],
                        adjb[:, g0 : g0 + H1], AL.add,
                    )
                    nc.gpsimd.tensor_tensor(
                        satt[:, g0 + H1 : g0 + GW], satt[:, g0 + H1 : g0 + GW],
                        adjb[:, g0 + H1 : g0 + GW], AL.add,
                    )
                    stage = psA.tile([P, GW], F16)
                    for k in range(GRP):
                        c = g * GRP + k
                        nc.tensor.matmul(
                            stage[:, k * P : (k + 1) * P],
                            satt[:, c * P : (c + 1) * P],
                            ident[:],
                            is_transpose=True, start=True, stop=True,
                        )
                    att = attp.tile([P, GW], F16)
                    if g == 7:
                        nc.scalar.copy(att[:], stage[:])
                    else:
                        nc.vector.tensor_copy(att[:], stage[:])
                    for k in range(GRP):
                        c = g * GRP + k
                        nc.tensor.matmul(
                            acc[:],
                            att[:, k * P : (k + 1) * P],
                            inp2b[:, c * D : (c + 1) * D],
                            start=(c == 0), stop=(c == NCHUNK - 1),
                        )
                ot = outp.tile([P, D], F32)
                nc.vector.tensor_scalar(ot[:], acc[:], 0.5, None, AL.mult)
                nc.sync.dma_start(out[b * P : (b + 1) * P, :], ot[:])

            for rep in range(reps):
                for b in range(NBLK):
                    emit_front(b)
                    if b >= 1:
                        emit_back(b - 1)
                emit_back(NBLK - 1)
                if rep + 1 < reps:
                    # re-arm adj prefetches for the next repetition
                    for _b in range(min(2, NBLK)):
                        adjb_t[_b] = adjp.tile([P, M], BF16, name="adjb", tag="adjb")
                        nc.gpsimd.dma_start(
                            adjb_t[_b][:], adj[_b * P : (_b + 1) * P, :]
                        )

    nc.compile()
    return nc


def _get_nc():
    global _CACHED
    if _CACHED is None:
        _CACHED = _build_kernel()
    return _CACHED


def kernel(input1, input2, adj, a1, a2):
    global LAST_EXEC_NS
    nc = _get_nc()

    a1bv = np.ascontiguousarray(np.broadcast_to(np.asarray(a1, np.float32).reshape(1, D), (P, D)))
    a2bv = np.ascontiguousarray(np.broadcast_to(np.asarray(a2, np.float32).reshape(1, D), (P, D)))
    ident = np.eye(P, dtype=np.float16)

    input1 = np.ascontiguousarray(input1, dtype=np.float32)
    input2 = np.ascontiguousarray(input2, dtype=np.float32)
    adj = np.ascontiguousarray(adj, dtype=np.float32)

    in_maps = []
    for c in range(NCORES):
        r0, r1 = c * ROWS, (c + 1) * ROWS
        in_maps.append(
            {
                "input1": input1[r0:r1],
                "input2": input2,
                "adj": adj[r0:r1],
                "a1b": a1bv,
                "a2b": a2bv,
                "identd": ident,
            }
        )

    trace = bool(os.environ.get("GAT_TRACE"))
    res = run_bass_kernel_spmd(nc, in_maps, core_ids=list(range(NCORES)), trace=trace)
    LAST_EXEC_NS = res.exec_time_ns
    outs = [res.results[c]["out"] for c in range(NCORES)]
    return np.concatenate(outs, axis=0).astype(np.float32)



# revision 43
# speedup vs baseline: 1.2051x; 1.2051x over previous
"""GAT-style attention kernel for Trainium2, 8 NeuronCores.

Reference computation (N=M=8192, D=256, f32):
    e1 = input1 @ a1; e2 = (input2 @ a2).T
    e  = leaky_relu(e1 + e2, 0.2)
    att = softmax(where(adj>0, e, -9e15), axis=1)
    att = att * adj.sum(1, keepdims=True)
    att = att*0.5 + adj*0.5
    out = att @ input2

Device math per row i (w_ij = exp(leaky_relu(e1_i + e2_j))):
    denom_i = sum_j adj_ij w_ij ; deg_i = sum_j adj_ij ; delta_i = deg_i/denom_i
    out_i = 0.5 * [ (delta_i * (adj.w) + adj) @ input2 ]_i

Sharding: rows of N across 8 cores (1024 each); input2/a1/a2 replicated.

Per-core pipeline, 8 blocks of (128 rows x 8192 cols):
    adjb = bf16(adj block)                        [SWDGE cast-DMA]
    mt   = adjb - 1, accum -> deg                 [DVE TS 4x]
    mt  += e2b' (e2/BIG, f16)                     [DVE TT 2x / Pool split]
    lr   = prelu(BIG*mt + e1_i, 0.2)              [ACT]
    num  = exp(lr) bf16, accum -> denom           [ACT]
    delta_i = deg_i/denom_i; diag = ident*delta   [DVE small]
    stage_c = num_c^T @ diag + adjb_c^T @ I       [PE f32 PSUM accum:
                                                   fuses delta-scale,
                                                   transpose, and adj-add]
    att  = f16(stage)                             [DVE PSUM evac]
    acc += att_c^T @ inp2f16_c                    [PE matmul]
    out  = 0.5 * acc                              [DVE]
"""

import os
import numpy as np

import concourse.bass as bass
import concourse.bacc as bacc
import concourse.tile as tile
from concourse import mybir
from concourse.bass_utils import run_bass_kernel_spmd

N, M, D = 8192, 8192, 256
NCORES = 8
ROWS = N // NCORES
P = 128
NBLK = ROWS // P  # 8
NCHUNK = M // P  # 64
BIG = 150.0
SLOPE = 0.2
GRP = 8  # transpose chunks per PSUM stage
GW = GRP * P  # 1024

F32 = mybir.dt.float32
BF16 = mybir.dt.bfloat16
F16 = mybir.dt.float16

# columns of the e2-add TT handled by DVE; the rest go to Pool
HD = 4096
HD0 = 7168  # block 0: DVE takes more (it is free after the e2 AMR stream)

LAST_EXEC_NS = None
_CACHED = None


def _build_kernel(reps=1):
    nc = bacc.Bacc("TRN2", target_bir_lowering=False, debug=False)

    inp1 = nc.dram_tensor("input1", [ROWS, D], F32, kind="ExternalInput").ap()
    inp2 = nc.dram_tensor("input2", [M, D], F32, kind="ExternalInput").ap()
    adj = nc.dram_tensor("adj", [ROWS, M], F32, kind="ExternalInput").ap()
    a1b = nc.dram_tensor("a1b", [P, D], F32, kind="ExternalInput").ap()
    a2b = nc.dram_tensor("a2b", [P, D], BF16, kind="ExternalInput").ap()
    identd = nc.dram_tensor("identd", [P, P], BF16, kind="ExternalInput").ap()
    out = nc.dram_tensor("out", [ROWS, D], F32, kind="ExternalOutput").ap()

    # DRAM bounces for flattening e2 (computed column-wise) into row order;
    # one per half so the second half's scatter has no false WAR dependency
    # on the first half's broadcast
    e2da = nc.dram_tensor("e2da", [1, M // 2], F32).ap()
    e2db = nc.dram_tensor("e2db", [1, M // 2], F32).ap()

    AL = mybir.AluOpType

    with tile.TileContext(nc) as tc:
        with (
            tc.tile_pool(name="persist", bufs=1) as persist,
            tc.tile_pool(name="setup", bufs=1) as setup,
            tc.tile_pool(name="small", bufs=5) as small,
            tc.tile_pool(name="adjp", bufs=4) as adjp,
            tc.tile_pool(name="mtp", bufs=2) as mtp,
            tc.tile_pool(name="nump", bufs=3) as nump,
            tc.tile_pool(name="attp", bufs=2) as attp,
            tc.tile_pool(name="outp", bufs=2) as outp,
            tc.tile_pool(name="psA", bufs=3, space="PSUM") as psA,
            tc.tile_pool(name="psO", bufs=2, space="PSUM") as psO,
        ):
            # ---------------- setup ----------------
            ident = persist.tile([P, P], BF16)
            nc.sync.dma_start(ident[:], identd[:])
            a1t = persist.tile([P, D], F32)
            nc.sync.dma_start(a1t[:], a1b[:])
            a2t = persist.tile([P, D], BF16)
            nc.sync.dma_start(a2t[:], a2b[:])

            inp2b = persist.tile([P, NCHUNK * D], BF16)
            e2col = persist.tile([P, NCHUNK], F32)
            e1col = persist.tile([P, NBLK], F32)
            adjb_t = {}

            # input2 batches: cast f32->f16 directly in the DMA, AMR for e2
            TB = 4  # tiles per batched DMA
            for g in range(NCHUNK // TB):
                src = inp2[g * TB * P : (g + 1) * TB * P, :]
                src = src.rearrange("(tb p) d -> p tb d", p=P, tb=TB)
                dst = inp2b[:, g * TB * D : (g + 1) * TB * D]
                dst = dst.rearrange("p (tb d) -> p tb d", tb=TB, d=D)
                nc.gpsimd.dma_start(dst, src)
                for k in range(TB):
                    t = g * TB + k
                    scr = small.tile([P, 1], F32, tag="amrdummy")
                    nc.vector.affine_mul_reduce(
                        out=scr[:].broadcast_to([P, D]),
                        accum_out=e2col[:, t : t + 1],
                        in0=inp2b[:, t * D : (t + 1) * D],
                        in1=a2t[:],
                        scale=1.0 / BIG,
                        bias=0.0,
                    )
                if g == 5:
                    # adj block 0 starts loading early; quartered so the
                    # e2 scatter/broadcast can slot into the DMA queue
                    adjb_t[0] = adjp.tile([P, M], BF16, name="adjb", tag="adjb")
                    for q0 in range(0, M, M // 4):
                        nc.gpsimd.dma_start(
                            adjb_t[0][:, q0 : q0 + M // 4],
                            adj[0:P, q0 : q0 + M // 4],
                        )
                if g == 7:
                    # first half of the e2 chain: scatter + broadcast of
                    # columns [0:M/2] as soon as their AMRs are done, so
                    # block 0's e2-add can start ~10us earlier
                    with tc.high_priority():
                        e2d_s1 = e2da.rearrange(
                            "one (t p) -> one p t", p=P, t=NCHUNK // 2
                        )
                        nc.sync.dma_start(e2d_s1[0], e2col[:, : NCHUNK // 2])
                        e2b = persist.tile([P, M], F16)
                        nc.gpsimd.dma_start(
                            e2b[:, : M // 2],
                            e2da[:, :].broadcast_to([P, M // 2]),
                        )

            # second half of the e2 chain (scatter via stride AP); high
            # priority so its DMA isn't queued behind adj prefetches
            with tc.high_priority():
                e2d_s2 = e2db.rearrange(
                    "one (t p) -> one p t", p=P, t=NCHUNK // 2
                )
                nc.sync.dma_start(e2d_s2[0], e2col[:, NCHUNK // 2 :])
                nc.gpsimd.dma_start(
                    e2b[:, M // 2 :],
                    e2db[:, :].broadcast_to([P, M // 2]),
                )

            # e1 (prelu bias): load both batches, but only block 0's AMR
            # runs now -- the rest are deferred to after front(0)'s DVE work
            # so they don't delay block 0's e2-add
            e1tmp = []
            for g1 in range(NBLK // TB):
                tmp1 = setup.tile([P, TB * D], F32, tag="itile")
                e1tmp.append(tmp1)
                src1 = inp1[g1 * TB * P : (g1 + 1) * TB * P, :].rearrange(
                    "(tb p) d -> p tb d", p=P, tb=TB
                )
                dst1 = tmp1[:].rearrange("p (tb d) -> p tb d", tb=TB, d=D)
                nc.sync.dma_start(dst1, src1)

            def emit_e1_amr(b1):
                tmp1 = e1tmp[b1 // TB]
                k1 = b1 % TB
                scr = small.tile([P, 1], F32, tag="amrdummy")
                nc.vector.affine_mul_reduce(
                    out=scr[:].broadcast_to([P, D]),
                    accum_out=e1col[:, b1 : b1 + 1],
                    in0=tmp1[:, k1 * D : (k1 + 1) * D],
                    in1=a1t[:],
                    scale=1.0,
                    bias=0.0,
                )

            for _b1 in range(NBLK):
                emit_e1_amr(_b1)

            # adj blocks 1,2: a dummy one-column write that reads e2b's
            # last column forces their DMAs behind the critical e2b
            # broadcast in the DMA queue
            for _pb in (1, 2):
                adjb_t[_pb] = adjp.tile([P, M], BF16, name="adjb", tag="adjb")
                nc.gpsimd.tensor_copy(
                    adjb_t[_pb][:, 0:1], e2b[:, M - 1 : M]
                )
                for q0 in range(0, M, M // 2):
                    nc.gpsimd.dma_start(
                        adjb_t[_pb][:, q0 : q0 + M // 2],
                        adj[_pb * P : (_pb + 1) * P, q0 : q0 + M // 2],
                    )

            # ---------------- main loop ----------------
            # Explicit per-engine schedule. Period k is bounded by ACT's
            # C1,C2,D (~14.4us). Streams per period k:
            #   ACT : C1(k), C2(k), D(k)
            #   DVE : delta(k-1), diag(k-1), ot(k-2), G(k-1, g0..g3),
            #         A(k+1), Bdve(k+1), G(k-1, g4..g7)
            #   Pool: adj-prefetch-gen(k+2), Bpool(k+1) pieces
            #   PE  : per g: MM1(k-1,g), MM2(k-1,g+3), MM3(k-1,g-1);
            #         then MM3(k-1,g7), MM2(k, g0..g2)
            #   SP  : out-dma(k-2)
            NG = NCHUNK // GRP  # 8 groups
            mt_t, num_t, deg_t, den_t, diag_t, diagN_t = {}, {}, {}, {}, {}, {}
            acc_t, att_t, stage_t = {}, {}, {}

            def emit_A(b):
                mt = mtp.tile([P, M], F16, tag="mt")
                mt_t[b] = mt
                deg = small.tile([P, 1], F32, tag="deg")
                deg_t[b] = deg
                nc.vector.tensor_scalar(
                    mt[:], adjb_t[b][:], -1.0, float(M), AL.add, AL.add,
                    accum_out=deg[:],
                )
                # diag(deg) for the num-side matmuls; off the critical chain
                dgN = small.tile([P, P], BF16, tag="dgN")
                diagN_t[b] = dgN
                nc.vector.tensor_scalar(dgN[:], ident[:], deg[:], None, AL.mult)

            def emit_Bdve(b, lo, hi):
                mt = mt_t[b]
                nc.vector.tensor_tensor(
                    mt[:, lo:hi], mt[:, lo:hi], e2b[:, lo:hi], AL.add
                )

            def emit_Bpool(b, lo, hi):
                mt = mt_t[b]
                nc.gpsimd.tensor_tensor(
                    mt[:, lo:hi], mt[:, lo:hi], e2b[:, lo:hi], AL.add
                )

            e1ch_t = {}

            def emit_bias_chain(b):
                # tiny ACT op: bias_b = Copy(den(b-1)*0 + e1col[:, b]); its
                # only purpose is to make C(b) depend on D(b-1) in ACT's own
                # stream, so the compile-time scheduler cannot reorder
                # next-block prelus before this block's exp
                e1ch = small.tile([P, 1], F32, tag="e1ch")
                e1ch_t[b] = e1ch
                if b >= 1 and (b - 1) in den_t:
                    nc.scalar.activation(
                        e1ch[:], den_t[b - 1][:],
                        mybir.ActivationFunctionType.Prelu,
                        bias=e1col[:, b : b + 1], scale=0.0, alpha=1.0,
                    )
                else:
                    nc.vector.tensor_scalar(
                        e1ch[:], e1col[:, b : b + 1], 1.0, None, AL.mult
                    )

            def emit_C(b, lo, hi):
                # prelu in place on mt (its only later reader is the exp)
                mt = mt_t[b]
                nc.scalar.activation(
                    mt[:, lo:hi], mt[:, lo:hi],
                    mybir.ActivationFunctionType.Prelu,
                    bias=e1ch_t[b][:], scale=BIG, alpha=SLOPE,
                )

            def emit_D(b):
                num = nump.tile([P, M], BF16, tag="num")
                num_t[b] = num
                den = small.tile([P, 1], F32, tag="denom")
                den_t[b] = den
                mt = mt_t.pop(b)
                nc.scalar.activation(
                    num[:], mt[:], mybir.ActivationFunctionType.Exp,
                    accum_out=den[:],
                )

            def emit_delta_diag(b):
                # diag(den) for the adj-side matmuls: the only op between
                # D(b) and the PE group matmuls -- no division needed here.
                # (att @ inp2 then accumulates den*z + deg*y; the final
                # output scale divides by den.)
                deg_t.pop(b)
                dgD = small.tile([P, P], BF16, tag="dgD")
                diag_t[b] = dgD
                nc.vector.tensor_scalar(dgD[:], ident[:], den_t[b][:], None, AL.mult)

            def emit_prefetch(b, quarters=1):
                if b >= NBLK or b in adjb_t:
                    return
                nx = adjp.tile([P, M], BF16, name="adjb", tag="adjb")
                adjb_t[b] = nx
                qw = M // quarters
                for q0 in range(0, M, qw):
                    nc.gpsimd.dma_start(
                        nx[:, q0 : q0 + qw], adj[b * P : (b + 1) * P, q0 : q0 + qw]
                    )

            def emit_MM2(b, g):
                # adj transposes scaled by diag(den) open each group's stage
                stage = psA.tile([P, GW], F32, name="stage")
                stage_t[(b, g)] = stage
                adjb = adjb_t[b]
                dgD = diag_t[b]
                # PSUM zero-regions are 2KB (4 f32 chunks): exactly one
                # start per zone, else later accumulations get zeroed
                for k in range(GRP):
                    c0 = g * GW + k * P
                    nc.tensor.matmul(
                        stage[:, k * P : (k + 1) * P],
                        adjb[:, c0 : c0 + P], dgD[:],
                        start=(k % 4 == 0), stop=False,
                        skip_group_check=True,
                    )

            def emit_MM1(b, g):
                # deg row-scale via diag(deg) accumulated onto the stage
                stage = stage_t[(b, g)]
                num = num_t[b]
                dgN = diagN_t[b]
                for k in range(GRP):
                    c0 = g * GW + k * P
                    nc.tensor.matmul(
                        stage[:, k * P : (k + 1) * P],
                        num[:, c0 : c0 + P], dgN[:],
                        start=False, stop=(k % 4 == 3),
                        skip_group_check=True,
                    )

            def emit_G(b, g):
                att = attp.tile([P, GW], BF16, tag="att")
                att_t[(b, g)] = att
                stage = stage_t.pop((b, g))
                nc.vector.tensor_copy(att[:], stage[:])

            def emit_MM3(b, g):
                if g == 0:
                    acc_t[b] = psO.tile([P, D], F32, name="acc")
                acc = acc_t[b]
                att = att_t.pop((b, g))
                for k in range(GRP):
                    c = g * GRP + k
                    nc.tensor.matmul(
                        acc[:],
                        att[:, k * P : (k + 1) * P],
                        inp2b[:, c * D : (c + 1) * D],
                        start=(c == 0), stop=(c == NCHUNK - 1),
                    )

            def emit_ot(b):
                # out = 0.5 * acc / den  (the division deferred from the
                # attention stage; recip runs here, far off the PE chain)
                rec = small.tile([P, 1], F32, tag="rec")
                nc.vector.reciprocal(rec[:], den_t[b][:])
                ot = outp.tile([P, D], F32)
                acc = acc_t.pop(b)
                nc.vector.tensor_scalar(ot[:], acc[:], rec[:], 0.5, AL.mult, AL.mult)
                nc.sync.dma_start(out[b * P : (b + 1) * P, :], ot[:])

            def release_back(b):
                adjb_t.pop(b)
                num_t.pop(b)
                diag_t.pop(b)
                diagN_t.pop(b)

            # ---- pre-loop: front(0) (B fully on DVE: it is free after the
            # AMR stream, and Pool would wait on the late e2b half anyway) ----
            emit_A(0)
            emit_Bdve(0, 0, M // 2)
            emit_Bdve(0, M // 2, M)
            emit_bias_chain(0)
            emit_C(0, 0, M // 2)
            emit_C(0, M // 2, M)
            emit_D(0)
            emit_A(1)
            emit_Bdve(1, 0, HD)
            emit_Bpool(1, HD, (HD + M) // 2)
            emit_Bpool(1, (HD + M) // 2, M)
            emit_prefetch(3)
            emit_bias_chain(1)

            # ---- steady-state periods ----
            for k in range(1, NBLK):
                bb = k - 1  # back block
                emit_delta_diag(bb)
                if k - 2 >= 0:
                    emit_ot(k - 2)
                emit_prefetch(k + 2)
                if k >= 2:
                    emit_bias_chain(k)
                emit_C(k, 0, M // 2)
                emit_C(k, M // 2, M)
                for g in range(NG):
                    emit_MM2(bb, g)
                    emit_MM1(bb, g)
                    emit_G(bb, g)
                    if g == 3 and k + 1 < NBLK:
                        emit_A(k + 1)
                    if g == 5 and k + 1 < NBLK:
                        emit_Bdve(k + 1, 0, HD)
                        emit_Bpool(k + 1, HD, (HD + M) // 2)
                        emit_Bpool(k + 1, (HD + M) // 2, M)
                    if g >= 1:
                        emit_MM3(bb, g - 1)
                emit_MM3(bb, NG - 1)
                emit_D(k)
                release_back(bb)

            # ---- drain: back(NBLK-1) ----
            bb = NBLK - 1
            emit_delta_diag(bb)
            emit_ot(NBLK - 2)
            for g in range(NG):
                emit_MM2(bb, g)
                emit_MM1(bb, g)
                emit_G(bb, g)
                if g >= 1:
                    emit_MM3(bb, g - 1)
            emit_MM3(bb, NG - 1)
            emit_ot(bb)
            release_back(bb)

    nc.compile()
    return nc


def _get_nc():
    global _CACHED
    if _CACHED is None:
        _CACHED = _build_kernel()
    return _CACHED


def kernel(input1, input2, adj, a1, a2):
    global LAST_EXEC_NS
    nc = _get_nc()

    import ml_dtypes
    a1bv = np.ascontiguousarray(
        np.broadcast_to(np.asarray(a1, np.float32).reshape(1, D), (P, D))
    )
    a2bv = np.ascontiguousarray(
        np.broadcast_to(np.asarray(a2, np.float32).reshape(1, D), (P, D))
    ).astype(ml_dtypes.bfloat16)
    try:
        import ml_dtypes

        identv = np.eye(P, dtype=ml_dtypes.bfloat16)
    except Exception:  # pragma: no cover
        identv = np.eye(P, dtype=np.float32)
    input1 = np.ascontiguousarray(input1, dtype=np.float32)
    input2 = np.ascontiguousarray(input2, dtype=np.float32)
    adj = np.ascontiguousarray(adj, dtype=np.float32)

    in_maps = []
    for c in range(NCORES):
        r0, r1 = c * ROWS, (c + 1) * ROWS
        in_maps.append(
            {
                "input1": input1[r0:r1],
                "input2": input2,
                "adj": adj[r0:r1],
                "a1b": a1bv,
                "a2b": a2bv,
                "identd": identv,
            }
        )

    trace = bool(os.environ.get("GAT_TRACE"))
    res = run_bass_kernel_spmd(nc, in_maps, core_ids=list(range(NCORES)), trace=trace)
    LAST_EXEC_NS = res.exec_time_ns
    outs = [res.results[c]["out"] for c in range(NCORES)]
    return np.concatenate(outs, axis=0).astype(np.float32)


# revision 54
# speedup vs baseline: 1.2092x; 1.0035x over previous
"""GAT-style attention kernel for Trainium2, 8 NeuronCores.

Reference computation (N=M=8192, D=256, f32):
    e1 = input1 @ a1; e2 = (input2 @ a2).T
    e  = leaky_relu(e1 + e2, 0.2)
    att = softmax(where(adj>0, e, -9e15), axis=1)
    att = att * adj.sum(1, keepdims=True)
    att = att*0.5 + adj*0.5
    out = att @ input2

Device math per row i (w_ij = exp(leaky_relu(e1_i + e2_j))):
    denom_i = sum_j adj_ij w_ij ; deg_i = sum_j adj_ij ; delta_i = deg_i/denom_i
    out_i = 0.5 * [ (delta_i * (adj.w) + adj) @ input2 ]_i

Sharding: rows of N across 8 cores (1024 each); input2/a1/a2 replicated.

Per-core pipeline, 8 blocks of (128 rows x 8192 cols):
    adjb = bf16(adj block)                        [SWDGE cast-DMA]
    mt   = adjb - 1, accum -> deg                 [DVE TS 4x]
    mt  += e2b' (e2/BIG, f16)                     [DVE TT 2x / Pool split]
    lr   = prelu(BIG*mt + e1_i, 0.2)              [ACT]
    num  = exp(lr) bf16, accum -> denom           [ACT]
    delta_i = deg_i/denom_i; diag = ident*delta   [DVE small]
    stage_c = num_c^T @ diag + adjb_c^T @ I       [PE f32 PSUM accum:
                                                   fuses delta-scale,
                                                   transpose, and adj-add]
    att  = f16(stage)                             [DVE PSUM evac]
    acc += att_c^T @ inp2f16_c                    [PE matmul]
    out  = 0.5 * acc                              [DVE]
"""

import os
import numpy as np

import concourse.bass as bass
import concourse.bacc as bacc
import concourse.tile as tile
from concourse import mybir
from concourse.bass_utils import run_bass_kernel_spmd

N, M, D = 8192, 8192, 256
NCORES = 8
ROWS = N // NCORES
P = 128
NBLK = ROWS // P  # 8
NCHUNK = M // P  # 64
BIG = 150.0
SLOPE = 0.2
GRP = 8  # transpose chunks per PSUM stage
GW = GRP * P  # 1024

F32 = mybir.dt.float32
BF16 = mybir.dt.bfloat16
F16 = mybir.dt.float16

# columns of the e2-add TT handled by DVE; the rest go to Pool
HD = 4096
HD0 = 7168  # block 0: DVE takes more (it is free after the e2 AMR stream)

LAST_EXEC_NS = None
_CACHED = None


def _build_kernel(reps=1):
    nc = bacc.Bacc("TRN2", target_bir_lowering=False, debug=False)

    inp1 = nc.dram_tensor("input1", [ROWS, D], F32, kind="ExternalInput").ap()
    inp2 = nc.dram_tensor("input2", [M, D], F32, kind="ExternalInput").ap()
    adj = nc.dram_tensor("adj", [ROWS, M], F32, kind="ExternalInput").ap()
    a1b = nc.dram_tensor("a1b", [P, D], F32, kind="ExternalInput").ap()
    a2b = nc.dram_tensor("a2b", [P, D], BF16, kind="ExternalInput").ap()
    identd = nc.dram_tensor("identd", [P, P], BF16, kind="ExternalInput").ap()
    out = nc.dram_tensor("out", [ROWS, D], F32, kind="ExternalOutput").ap()

    # DRAM bounces for flattening e2 (computed column-wise) into row order;
    # one per half so the second half's scatter has no false WAR dependency
    # on the first half's broadcast
    e2da = nc.dram_tensor("e2da", [1, M // 2], F32).ap()
    e2db = nc.dram_tensor("e2db", [1, M // 2], F32).ap()

    AL = mybir.AluOpType

    with tile.TileContext(nc) as tc:
        with (
            tc.tile_pool(name="persist", bufs=1) as persist,
            tc.tile_pool(name="setup", bufs=1) as setup,
            tc.tile_pool(name="small", bufs=5) as small,
            tc.tile_pool(name="adjp", bufs=4) as adjp,
            tc.tile_pool(name="mtp", bufs=2) as mtp,
            tc.tile_pool(name="nump", bufs=3) as nump,
            tc.tile_pool(name="attp", bufs=2) as attp,
            tc.tile_pool(name="outp", bufs=2) as outp,
            tc.tile_pool(name="psA", bufs=3, space="PSUM") as psA,
            tc.tile_pool(name="psO", bufs=2, space="PSUM") as psO,
        ):
            # ---------------- setup ----------------
            ident = persist.tile([P, P], BF16)
            nc.sync.dma_start(ident[:], identd[:])
            a1t = persist.tile([P, D], F32)
            nc.sync.dma_start(a1t[:], a1b[:])
            a2t = persist.tile([P, D], BF16)
            nc.sync.dma_start(a2t[:], a2b[:])

            inp2b = persist.tile([P, NCHUNK * D], BF16)
            e2col = persist.tile([P, NCHUNK], F32)
            e1col = persist.tile([P, NBLK], F32)
            adjb_t = {}

            # input2 batches: cast f32->f16 directly in the DMA, AMR for e2
            TB = 4  # tiles per batched DMA
            for g in range(NCHUNK // TB):
                src = inp2[g * TB * P : (g + 1) * TB * P, :]
                src = src.rearrange("(tb p) d -> p tb d", p=P, tb=TB)
                dst = inp2b[:, g * TB * D : (g + 1) * TB * D]
                dst = dst.rearrange("p (tb d) -> p tb d", tb=TB, d=D)
                nc.gpsimd.dma_start(dst, src)
                for k in range(TB):
                    t = g * TB + k
                    scr = small.tile([P, 1], F32, tag="amrdummy")
                    nc.vector.affine_mul_reduce(
                        out=scr[:].broadcast_to([P, D]),
                        accum_out=e2col[:, t : t + 1],
                        in0=inp2b[:, t * D : (t + 1) * D],
                        in1=a2t[:],
                        scale=1.0 / BIG,
                        bias=0.0,
                    )
                if g == 5:
                    # adj block 0 starts loading early; quartered so the
                    # e2 scatter/broadcast can slot into the DMA queue
                    adjb_t[0] = adjp.tile([P, M], BF16, name="adjb", tag="adjb")
                    for q0 in range(0, M, M // 4):
                        nc.gpsimd.dma_start(
                            adjb_t[0][:, q0 : q0 + M // 4],
                            adj[0:P, q0 : q0 + M // 4],
                        )
                if g == 7:
                    # first half of the e2 chain: scatter + broadcast of
                    # columns [0:M/2] as soon as their AMRs are done, so
                    # block 0's e2-add can start ~10us earlier
                    with tc.high_priority():
                        e2d_s1 = e2da.rearrange(
                            "one (t p) -> one p t", p=P, t=NCHUNK // 2
                        )
                        nc.sync.dma_start(e2d_s1[0], e2col[:, : NCHUNK // 2])
                        e2b = persist.tile([P, M], F16)
                        nc.gpsimd.dma_start(
                            e2b[:, : M // 2],
                            e2da[:, :].broadcast_to([P, M // 2]),
                        )

            # second half of the e2 chain (scatter via stride AP); high
            # priority so its DMA isn't queued behind adj prefetches
            with tc.high_priority():
                e2d_s2 = e2db.rearrange(
                    "one (t p) -> one p t", p=P, t=NCHUNK // 2
                )
                nc.sync.dma_start(e2d_s2[0], e2col[:, NCHUNK // 2 :])
                nc.gpsimd.dma_start(
                    e2b[:, M // 2 :],
                    e2db[:, :].broadcast_to([P, M // 2]),
                )

            # e1 (prelu bias): load both batches, but only block 0's AMR
            # runs now -- the rest are deferred to after front(0)'s DVE work
            # so they don't delay block 0's e2-add
            e1tmp = []
            for g1 in range(NBLK // TB):
                tmp1 = setup.tile([P, TB * D], F32, tag="itile")
                e1tmp.append(tmp1)
                src1 = inp1[g1 * TB * P : (g1 + 1) * TB * P, :].rearrange(
                    "(tb p) d -> p tb d", p=P, tb=TB
                )
                dst1 = tmp1[:].rearrange("p (tb d) -> p tb d", tb=TB, d=D)
                nc.sync.dma_start(dst1, src1)

            def emit_e1_amr(b1):
                tmp1 = e1tmp[b1 // TB]
                k1 = b1 % TB
                scr = small.tile([P, 1], F32, tag="amrdummy")
                nc.vector.affine_mul_reduce(
                    out=scr[:].broadcast_to([P, D]),
                    accum_out=e1col[:, b1 : b1 + 1],
                    in0=tmp1[:, k1 * D : (k1 + 1) * D],
                    in1=a1t[:],
                    scale=1.0,
                    bias=0.0,
                )

            for _b1 in range(NBLK):
                emit_e1_amr(_b1)

            # adj blocks 1,2: a dummy one-column write that reads e2b's
            # last column forces their DMAs behind the critical e2b
            # broadcast in the DMA queue
            for _pb in (1, 2):
                adjb_t[_pb] = adjp.tile([P, M], BF16, name="adjb", tag="adjb")
                nc.gpsimd.tensor_copy(
                    adjb_t[_pb][:, 0:1], e2b[:, M - 1 : M]
                )
                for q0 in range(0, M, M // 2):
                    nc.gpsimd.dma_start(
                        adjb_t[_pb][:, q0 : q0 + M // 2],
                        adj[_pb * P : (_pb + 1) * P, q0 : q0 + M // 2],
                    )

            # ---------------- main loop ----------------
            # Explicit per-engine schedule. Period k is bounded by ACT's
            # C1,C2,D (~14.4us). Streams per period k:
            #   ACT : C1(k), C2(k), D(k)
            #   DVE : delta(k-1), diag(k-1), ot(k-2), G(k-1, g0..g3),
            #         A(k+1), Bdve(k+1), G(k-1, g4..g7)
            #   Pool: adj-prefetch-gen(k+2), Bpool(k+1) pieces
            #   PE  : per g: MM1(k-1,g), MM2(k-1,g+3), MM3(k-1,g-1);
            #         then MM3(k-1,g7), MM2(k, g0..g2)
            #   SP  : out-dma(k-2)
            NG = NCHUNK // GRP  # 8 groups
            mt_t, num_t, deg_t, den_t, diag_t, diagN_t = {}, {}, {}, {}, {}, {}
            acc_t, att_t, stage_t = {}, {}, {}

            def emit_A(b):
                mt = mtp.tile([P, M], F16, tag="mt")
                mt_t[b] = mt
                deg = small.tile([P, 1], F32, tag="deg")
                deg_t[b] = deg
                nc.vector.tensor_scalar(
                    mt[:], adjb_t[b][:], -1.0, float(M), AL.add, AL.add,
                    accum_out=deg[:],
                )
                # diag(deg) for the num-side matmuls; off the critical chain
                dgN = small.tile([P, P], BF16, tag="dgN")
                diagN_t[b] = dgN
                nc.vector.tensor_scalar(dgN[:], ident[:], deg[:], None, AL.mult)

            def emit_Bdve(b, lo, hi):
                mt = mt_t[b]
                nc.vector.tensor_tensor(
                    mt[:, lo:hi], mt[:, lo:hi], e2b[:, lo:hi], AL.add
                )

            def emit_Bpool(b, lo, hi):
                mt = mt_t[b]
                nc.gpsimd.tensor_tensor(
                    mt[:, lo:hi], mt[:, lo:hi], e2b[:, lo:hi], AL.add
                )

            e1ch_t = {}

            def emit_bias_chain(b):
                # tiny ACT op: bias_b = Copy(den(b-1)*0 + e1col[:, b]); its
                # only purpose is to make C(b) depend on D(b-1) in ACT's own
                # stream, so the compile-time scheduler cannot reorder
                # next-block prelus before this block's exp
                e1ch = small.tile([P, 1], F32, tag="e1ch")
                e1ch_t[b] = e1ch
                if b >= 1 and (b - 1) in den_t:
                    nc.scalar.activation(
                        e1ch[:], den_t[b - 1][:],
                        mybir.ActivationFunctionType.Prelu,
                        bias=e1col[:, b : b + 1], scale=0.0, alpha=1.0,
                    )
                else:
                    nc.vector.tensor_scalar(
                        e1ch[:], e1col[:, b : b + 1], 1.0, None, AL.mult
                    )

            def emit_C(b, lo, hi):
                # prelu in place on mt (its only later reader is the exp)
                mt = mt_t[b]
                nc.scalar.activation(
                    mt[:, lo:hi], mt[:, lo:hi],
                    mybir.ActivationFunctionType.Prelu,
                    bias=e1ch_t[b][:], scale=BIG, alpha=SLOPE,
                )

            def emit_D(b):
                num = nump.tile([P, M], BF16, tag="num")
                num_t[b] = num
                den = small.tile([P, 1], F32, tag="denom")
                den_t[b] = den
                mt = mt_t.pop(b)
                nc.scalar.activation(
                    num[:], mt[:], mybir.ActivationFunctionType.Exp,
                    accum_out=den[:],
                )

            def emit_delta_diag(b):
                # diag(den) for the adj-side matmuls: the only op between
                # D(b) and the PE group matmuls -- no division needed here.
                # (att @ inp2 then accumulates den*z + deg*y; the final
                # output scale divides by den.)
                deg_t.pop(b)
                dgD = small.tile([P, P], BF16, tag="dgD")
                diag_t[b] = dgD
                nc.vector.tensor_scalar(dgD[:], ident[:], den_t[b][:], None, AL.mult)

            def emit_prefetch(b, quarters=1):
                if b >= NBLK or b in adjb_t:
                    return
                nx = adjp.tile([P, M], BF16, name="adjb", tag="adjb")
                adjb_t[b] = nx
                qw = M // quarters
                for q0 in range(0, M, qw):
                    nc.gpsimd.dma_start(
                        nx[:, q0 : q0 + qw], adj[b * P : (b + 1) * P, q0 : q0 + qw]
                    )

            def emit_MM2(b, g):
                # adj transposes scaled by diag(den) open each group's stage
                stage = psA.tile([P, GW], F32, name="stage")
                stage_t[(b, g)] = stage
                adjb = adjb_t[b]
                dgD = diag_t[b]
                # PSUM zero-regions are 2KB (4 f32 chunks): exactly one
                # start per zone, else later accumulations get zeroed
                for k in range(GRP):
                    c0 = g * GW + k * P
                    nc.tensor.matmul(
                        stage[:, k * P : (k + 1) * P],
                        adjb[:, c0 : c0 + P], dgD[:],
                        start=(k % 4 == 0), stop=False,
                        skip_group_check=True,
                    )

            def emit_MM1(b, g):
                # deg row-scale via diag(deg) accumulated onto the stage
                stage = stage_t[(b, g)]
                num = num_t[b]
                dgN = diagN_t[b]
                for k in range(GRP):
                    c0 = g * GW + k * P
                    nc.tensor.matmul(
                        stage[:, k * P : (k + 1) * P],
                        num[:, c0 : c0 + P], dgN[:],
                        start=False, stop=(k % 4 == 3),
                        skip_group_check=True,
                    )

            def emit_G(b, g):
                att = attp.tile([P, GW], BF16, tag="att")
                att_t[(b, g)] = att
                stage = stage_t.pop((b, g))
                nc.vector.tensor_copy(att[:], stage[:])

            def emit_MM3(b, g):
                if g == 0:
                    acc_t[b] = psO.tile([P, D], F32, name="acc")
                acc = acc_t[b]
                att = att_t.pop((b, g))
                for k in range(GRP):
                    c = g * GRP + k
                    nc.tensor.matmul(
                        acc[:],
                        att[:, k * P : (k + 1) * P],
                        inp2b[:, c * D : (c + 1) * D],
                        start=(c == 0), stop=(c == NCHUNK - 1),
                    )

            def emit_ot(b):
                # out = 0.5 * acc / den  (the division deferred from the
                # attention stage; recip runs here, far off the PE chain)
                rec = small.tile([P, 1], F32, tag="rec")
                nc.vector.reciprocal(rec[:], den_t[b][:])
                ot = outp.tile([P, D], F32)
                acc = acc_t.pop(b)
                nc.vector.tensor_scalar(ot[:], acc[:], rec[:], 0.5, AL.mult, AL.mult)
                nc.sync.dma_start(out[b * P : (b + 1) * P, :], ot[:])

            def release_back(b):
                adjb_t.pop(b)
                num_t.pop(b)
                diag_t.pop(b)
                diagN_t.pop(b)

            # ---- pre-loop: front(0) (B fully on DVE: it is free after the
            # AMR stream, and Pool would wait on the late e2b half anyway) ----
            emit_A(0)
            emit_Bdve(0, 0, M // 2)
            emit_Bdve(0, M // 2, M)
            emit_bias_chain(0)
            emit_C(0, 0, M // 2)
            emit_C(0, M // 2, M)
            emit_D(0)
            emit_A(1)
            # block 1: DVE is idle until D(0) completes, so it takes most of
            # the e2-add; Pool only gets the tail
            emit_Bdve(1, 0, HD)
            emit_Bdve(1, HD, HD + 2048)
            emit_Bpool(1, HD + 2048, M)
            emit_prefetch(3)
            emit_bias_chain(1)

            # ---- steady-state periods ----
            for k in range(1, NBLK):
                bb = k - 1  # back block
                emit_delta_diag(bb)
                if k - 2 >= 0:
                    emit_ot(k - 2)
                emit_prefetch(k + 2)
                if k >= 2:
                    emit_bias_chain(k)
                emit_C(k, 0, M // 2)
                emit_C(k, M // 2, M)
                for g in range(NG):
                    emit_MM2(bb, g)
                    emit_MM1(bb, g)
                    emit_G(bb, g)
                    if g == 3 and k + 1 < NBLK:
                        emit_A(k + 1)
                    if g == 5 and k + 1 < NBLK:
                        emit_Bdve(k + 1, 0, HD)
                        emit_Bpool(k + 1, HD, (HD + M) // 2)
                        emit_Bpool(k + 1, (HD + M) // 2, M)
                    if g >= 1:
                        emit_MM3(bb, g - 1)
                emit_MM3(bb, NG - 1)
                emit_D(k)
                release_back(bb)

            # ---- drain: back(NBLK-1) ----
            bb = NBLK - 1
            emit_delta_diag(bb)
            emit_ot(NBLK - 2)
            for g in range(NG):
                emit_MM2(bb, g)
                emit_MM1(bb, g)
                emit_G(bb, g)
                if g >= 1:
                    emit_MM3(bb, g - 1)
            emit_MM3(bb, NG - 1)
            emit_ot(bb)
            release_back(bb)

    nc.compile()
    return nc


def _get_nc():
    global _CACHED
    if _CACHED is None:
        _CACHED = _build_kernel()
    return _CACHED


def kernel(input1, input2, adj, a1, a2):
    global LAST_EXEC_NS
    nc = _get_nc()

    import ml_dtypes
    a1bv = np.ascontiguousarray(
        np.broadcast_to(np.asarray(a1, np.float32).reshape(1, D), (P, D))
    )
    a2bv = np.ascontiguousarray(
        np.broadcast_to(np.asarray(a2, np.float32).reshape(1, D), (P, D))
    ).astype(ml_dtypes.bfloat16)
    try:
        import ml_dtypes

        identv = np.eye(P, dtype=ml_dtypes.bfloat16)
    except Exception:  # pragma: no cover
        identv = np.eye(P, dtype=np.float32)
    input1 = np.ascontiguousarray(input1, dtype=np.float32)
    input2 = np.ascontiguousarray(input2, dtype=np.float32)
    adj = np.ascontiguousarray(adj, dtype=np.float32)

    in_maps = []
    for c in range(NCORES):
        r0, r1 = c * ROWS, (c + 1) * ROWS
        in_maps.append(
            {
                "input1": input1[r0:r1],
                "input2": input2,
                "adj": adj[r0:r1],
                "a1b": a1bv,
                "a2b": a2bv,
                "identd": identv,
            }
        )

    trace = bool(os.environ.get("GAT_TRACE"))
    res = run_bass_kernel_spmd(nc, in_maps, core_ids=list(range(NCORES)), trace=trace)
    LAST_EXEC_NS = res.exec_time_ns
    outs = [res.results[c]["out"] for c in range(NCORES)]
    return np.concatenate(outs, axis=0).astype(np.float32)
